# revision 72
# baseline (speedup 1.0000x reference)
"""Trainium2 Bass kernel for nn_Memory (GRU-style scan over 16384 rows, d=512).

Strategy: the recurrence m_t = (1-z_t) m_{t-1} + z_t h_t is *linear in m given
the gates*, and the gates depend on m_{t-1} through two 512x512 matvecs. Each
block is solved by Picard iteration: compute all gates from the previous
iterate's shifted states with large batched matmuls (full PE utilization),
then re-propagate the states exactly with the hardware linear scan primitive
(tensor_tensor_scan). In-pass feature-chunk ordering makes this Gauss-Seidel,
converging at ~0.5-0.6x error per pass.

Default (8-core single-exchange scheme, build_kernel8): each core owns a
2048-row block. The GRU dynamics forget the block-initial state within ~48
rows (numpy: state diff 4e-6 by t=48), so each block's FINAL state is
independent of its incoming carry. Each core therefore converges its block
locally with a zero carry (the az-only init pass is fused into phase 1 as a
second activation off the same x.W psum, then 4 passes with fp8e4 DoubleRow
U-matmuls at 0.5 cycles/row and 4 fp16 passes), ONE tiny AllGather
(issued after pass 7, overlapping the final pass) distributes the block-final
states, and only the first 64 rows are re-solved with the true carry (10
chunk-Jacobi window passes, double-buffered so all 4 feature chunks pipeline)
while the tail rows stream out between window passes. Measured on hw:
rel L2 ~2.5e-3, max rel ~8.8e-3 (tolerance 2e-2).

Per core and per full pass: az injection via fp16 identity matmul + U-matvec
accumulation against stationary [Uz|Uh] tiles; sigmoid/tanh on ACT straight
out of PSUM; d0=1-z, d1=z*h and the chained 512-wide scans on DVE (fp32 scan
accumulator; passes feeding a DoubleRow pass scan straight to the fp8 shadow
state, pass 4 refreshes the fp16 state for the fp16 passes, and the final
pass + output store fp16 for 1-cycle/row transposes).

Host-side prep (free, like the weight packing): x is transposed/cast to fp16
partition-major [128, tb, k, 512] so phase 1 needs no PE transposes, and
W/U/U8 are packed partition-major so every weight DMA is one contiguous run
per partition (HWDGE dispatch is ~630ns per DMA and serial at startup; only
the W z-plane + bias gate the first matmul).

MEMORY_KERNEL_MODE=single selects the legacy single-core variant
(build_kernel); it predates the packed host layouts and is retained for
reference only.
"""

import sys

sys.path.insert(0, "/opt/trn_rl_repo")

import numpy as np

import concourse.bass as bass
import concourse.mybir as mybir
import concourse.tile as tile
from concourse.bass import ds
from concourse.bass_utils import run_bass_kernel_spmd

T = 16384
D = 512  # in/out features
DO = 2 * D  # packed gate outputs (z | h)
B = 2048  # fixed-point block length
NBLK = T // B
NPASS = 17  # gate/scan passes per block (pass 0: no U-matmul; last: fp32 scan)
KCH = D // 128  # 4 contraction chunks
JCH = DO // 128  # 8 output chunks (0..3 -> z, 4..7 -> h)
NSUB = B // 512  # 512-column matmul subtiles per block

FP32 = mybir.dt.float32
FP16 = mybir.dt.float16
FP8 = mybir.dt.float8e4
AF = mybir.ActivationFunctionType
ALU = mybir.AluOpType


def _apply_tile_drain_patch():
    """This container's walrus rejects >1 sync-wait on the TileContext exit
    Drain (setupSyncWait/CTRL_NO_STRUCT). Split the accumulated end-of-kernel
    waits into one Drain per semaphore."""
    import bass_rust

    def _drain_and_barrier(self, tick_clock, wait_clock):
        drain_inst = self.nc.sync.drain()
        wait_clock.add_sem_waits(
            drain_inst.ins, tile.ScopedClock({None: tick_clock.global_clock})
        )
        si = drain_inst.ins.sync_info
        if si is not None and len(si.on_wait) > 1:
            waits = list(si.on_wait)
            si.on_wait = waits[:1]
            for w in waits[1:]:
                d2 = self.nc.sync.drain()
                s2 = d2.ins.sync_info
                if s2 is None:
                    d2.ins.sync_info = bass_rust.SyncInfo(on_wait=[w], on_update=[])
                else:
                    s2.on_wait = [w]
        self.nc.all_engine_barrier()
        assert self.sems is not None
        popped = self.nc._tile_sem_poison_stack.pop()
        assert popped is self._sem_poison
        self.nc.clear_and_free_semaphores(list(self.sems.allocated().values()))
        self.nc.all_engine_barrier()

    tile.TileContext._drain_and_barrier = _drain_and_barrier


def _split_multi_waits(nc):
    """This walrus build encodes at most ONE sync-wait per hardware
    instruction. Hoist extra waits onto same-engine NoOps placed immediately
    before the owning instruction (engines execute block order, so the waits
    still all complete before it runs)."""
    import bass_rust

    nid = 0
    for f in nc.m.functions:
        for b in f.blocks:
            out = []
            changed = False
            for ins in b.instructions:
                si = ins.sync_info
                if si is not None and len(si.on_wait) > 1:
                    waits = list(si.on_wait)
                    for w in waits[:-1]:
                        nop = mybir.InstNoOp(name=f"I-waitsplit-{nid}", ins=[], outs=[])
                        nid += 1
                        nop.engine = ins.engine
                        nop.sync_info = bass_rust.SyncInfo(on_wait=[w], on_update=[])
                        out.append(nop)
                    si.on_wait = waits[-1:]
                    changed = True
                out.append(ins)
            if changed:
                b.instructions = out


def build_kernel(npass=NPASS, phase1=True, fixpoint=True):
    _apply_tile_drain_patch()
    nc = bass.Bass("TRN2")

    x = nc.dram_tensor("x", [T, D], FP32, kind="ExternalInput")
    wp = nc.dram_tensor("wp", [D, DO], FP16, kind="ExternalInput")  # [Wz|Wh]
    up = nc.dram_tensor("up", [D, DO], FP16, kind="ExternalInput")  # [Uz|Uh]
    i16 = nc.dram_tensor("i16", [128, 128], FP16, kind="ExternalInput")
    i32 = nc.dram_tensor("i32", [128, 128], FP32, kind="ExternalInput")
    bp = nc.dram_tensor("bp", [128, JCH], FP32, kind="ExternalInput")  # bias chunks
    ys = nc.dram_tensor("ys", [T, D], FP32, kind="ExternalOutput")

    with tile.TileContext(nc) as tc:
        consts = tc.alloc_tile_pool(name="consts", bufs=1)
        usb = consts.tile([128, KCH, DO], FP16, tag="usb")
        wsb = consts.tile([128, JCH, KCH, 128], FP16, tag="wsb")
        id16 = consts.tile([128, 128], FP16, tag="id16")
        id32 = consts.tile([128, 128], FP32, tag="id32")
        bsb = consts.tile([128, JCH], FP32, tag="bsb")
        nc.sync.dma_start(usb[:], up.rearrange("(k p) m -> p k m", p=128))
        nc.sync.dma_start(wsb[:], wp.rearrange("(k p) m -> p k m", p=128))
        nc.sync.dma_start(id16[:], i16[:])
        nc.sync.dma_start(id32[:], i32[:])
        nc.sync.dma_start(bsb[:], bp[:])

        dram = tc.alloc_tile_pool(name="dram", bufs=1, space="DRAM")
        # AZ^T/AH^T staged as [feat%128, out-chunk j, t]; j<4: z, j>=4: h
        azt = dram.tile([128, JCH, T], FP16, tag="azt")

        # ---------------- phase 1: x^T and AZ/AH ----------------
        with (
            tc.tile_pool(name="p1", bufs=3) as p1,
            tc.tile_pool(name="p1ps", bufs=4, space="PSUM") as p1ps,
            tc.tile_pool(name="p1az", bufs=2, space="PSUM") as p1az,
        ):
            for tb in range(T // 512 if phase1 else 0):
                xT = p1.tile([128, KCH, 512], FP16, tag="xT")
                for s in range(4):
                    xt = p1.tile([128, D], FP32, tag="xt")
                    t0 = tb * 512 + s * 128
                    nc.sync.dma_start(xt[:], x[t0 : t0 + 128, :])
                    for k in range(KCH):
                        pst = p1ps.tile([128, 128], FP32, tag="pst")
                        nc.tensor.transpose(
                            pst[:], xt[:, k * 128 : (k + 1) * 128], id32[:]
                        )
                        nc.vector.tensor_copy(
                            xT[:, k, s * 128 : (s + 1) * 128], pst[:]
                        )
                az16 = p1.tile([128, JCH, 512], FP16, tag="az16")
                for j in range(JCH):
                    psa = p1az.tile([128, 512], FP32, tag="psa")
                    for k in range(KCH):
                        nc.tensor.matmul(
                            psa[:],
                            wsb[:, j, k, :],
                            xT[:, k, :],
                            start=(k == 0),
                            stop=(k == KCH - 1),
                        )
                    # az16 = psum + bias_chunk (per-partition), cast fp16
                    nc.scalar.activation(
                        az16[:, j, :], psa[:], AF.Identity, bias=bsb[:, j : j + 1]
                    )
                nc.sync.dma_start(azt[:, :, tb * 512 : (tb + 1) * 512], az16[:])

        # ---------------- phase 2: blockwise fixed point ----------------
        with (
            tc.tile_pool(name="st", bufs=1) as st,
            tc.tile_pool(name="gates", bufs=1) as gates,
            tc.tile_pool(name="az2", bufs=1) as az2,
            tc.tile_pool(name="carry", bufs=2) as carryp,
            tc.tile_pool(name="outs", bufs=4) as outs,
            tc.tile_pool(name="ps2", bufs=6, space="PSUM") as ps2,
            tc.tile_pool(name="pst2", bufs=2, space="PSUM") as pst2,
        ):
            carry = carryp.tile([128, KCH], FP32, tag="carry")
            nc.vector.memset(carry[:], 0.0)

            for b in range(NBLK if fixpoint else 0):
                azb = az2.tile([128, JCH, B], FP16, tag="azb")
                nc.sync.dma_start(azb[:], azt[:, :, b * B : (b + 1) * B])

                # states, shifted by one: col 0 = carry, cols 1.. = m_t
                mx = st.tile([128, KCH, B + 1], FP16, tag="mx")
                m32 = st.tile([128, KCH, B], FP32, tag="m32")
                for c in range(KCH):
                    nc.vector.tensor_copy(mx[:, c, 0:1], carry[:, c : c + 1])

                zt = gates.tile([128, KCH, B], FP16, tag="zt")
                ht = gates.tile([128, KCH, B], FP16, tag="ht")
                d0 = gates.tile([128, KCH, B], FP16, tag="d0")
                d1 = gates.tile([128, KCH, B], FP16, tag="d1")

                for p in range(npass):
                    first = p == 0
                    last = p == npass - 1
                    for c in range(KCH):
                        for j in (c, c + KCH):  # z-chunk then h-chunk
                            for s in range(NSUB):
                                psg = ps2.tile([128, 512], FP32, tag="psg")
                                nc.tensor.matmul(
                                    psg[:],
                                    id16[:],
                                    azb[:, j, s * 512 : (s + 1) * 512],
                                    start=True,
                                    stop=first,
                                )
                                if not first:
                                    for k in range(KCH):
                                        nc.tensor.matmul(
                                            psg[:],
                                            usb[:, k, j * 128 : (j + 1) * 128],
                                            mx[:, k, s * 512 : s * 512 + 512],
                                            start=False,
                                            stop=(k == KCH - 1),
                                        )
                                dst = zt if j < KCH else ht
                                fn = AF.Sigmoid if j < KCH else AF.Tanh
                                nc.scalar.activation(
                                    dst[:, c, s * 512 : (s + 1) * 512], psg[:], fn
                                )
                        # d0 = 1 - z ; d1 = z * h
                        nc.vector.tensor_scalar(
                            d0[:, c, :], zt[:, c, :], -1.0, 1.0, ALU.mult, ALU.add
                        )
                        nc.vector.tensor_mul(d1[:, c, :], zt[:, c, :], ht[:, c, :])
                        # m_t = d0_t * m_{t-1} + d1_t  (exact sequential scan)
                        out_ap = m32[:, c, :] if last else mx[:, c, 1 : B + 1]
                        nc.vector.tensor_tensor_scan(
                            out_ap,
                            d0[:, c, :],
                            d1[:, c, :],
                            carry[:, c : c + 1],
                            ALU.mult,
                            ALU.add,
                        )

                ncarry = carryp.tile([128, KCH], FP32, tag="carry")
                for c in range(KCH):
                    nc.vector.tensor_copy(ncarry[:, c : c + 1], m32[:, c, B - 1 : B])
                carry = ncarry

                # transpose states back to [t, feat] rows and store
                for tt in range(B // 128):
                    yst = outs.tile([128, D], FP32, tag="yst")
                    for c in range(KCH):
                        psy = pst2.tile([128, 128], FP32, tag="psy")
                        nc.tensor.transpose(
                            psy[:], m32[:, c, tt * 128 : (tt + 1) * 128], id32[:]
                        )
                        nc.vector.tensor_copy(
                            yst[:, c * 128 : (c + 1) * 128], psy[:]
                        )
                    t0 = b * B + tt * 128
                    nc.sync.dma_start(ys[t0 : t0 + 128, :], yst[:])

        consts.release()
        dram.release()

    _split_multi_waits(nc)
    return nc



NCORE = 8
BC = T // NCORE  # rows per core in the 8-core kernel
NPASS8 = 9  # full-block passes (pass 0: az-only init + 8 Picard sweeps)
N8 = 4  # of which: passes 1..N8 use fp8e4 DoubleRow U-matmuls (4x PE rate)
NWARM = 14  # startup PE p-state warm-up matmuls (dependency-free)
NSUB8 = BC // 512
FIXW = 64  # head-window length re-solved after the single carry exchange
FIXP = 10  # fixup passes on the head window (chunk-Jacobi, double-buffered)


def build_kernel8(npass=NPASS8, sim_nocc=False, fixw=FIXW, fixp=FIXP,
                  wide_act=False, korder_rot=False, fix_jacobi=True,
                  psbufs=None, n8=N8, nwarm=NWARM):
    # sim_nocc: drop the AllGather (TimelineSim can't model collectives) so the
    # per-core occupancy can be cost-model-simulated; numerics become wrong.
    #
    # Single-exchange scheme: the GRU state forgets its block-initial carry
    # within ~48 steps (state diff 4e-6 by t=48), so each core's final state is
    # independent of its incoming carry. All full passes therefore run with a
    # zero carry; ONE AllGather (issued after pass npass-2, overlapping the
    # last pass) distributes the block-final states, and only the first `fixw`
    # rows are re-solved with the true carry (`fixp` cheap window passes).
    _apply_tile_drain_patch()
    nc = bass.Bass("TRN2", num_devices=NCORE)

    # host-packed partition-major layouts: one contiguous run per partition
    # keeps DMA descriptor counts (and SEQ dispatch time) minimal
    xt = nc.dram_tensor("xt", [128, BC // 512, KCH, 512], FP16, kind="ExternalInput")
    up8 = nc.dram_tensor("up8", [128, KCH * DO], FP8, kind="ExternalInput")
    wp = nc.dram_tensor("wp", [128, JCH, KCH, 128], FP16, kind="ExternalInput")
    up = nc.dram_tensor("up", [128, KCH * DO], FP16, kind="ExternalInput")
    i16 = nc.dram_tensor("i16", [128, 128], FP16, kind="ExternalInput")
    i32 = nc.dram_tensor("i32", [128, 128], FP32, kind="ExternalInput")
    bp = nc.dram_tensor("bp", [128, JCH], FP32, kind="ExternalInput")
    ys = nc.dram_tensor("ys", [BC, D], FP32, kind="ExternalOutput")

    cin = nc.dram_tensor("cin", [1, D], FP32)
    gath9 = nc.dram_tensor("gath9", [NCORE + 1, D], FP32, addr_space="Shared")

    with tile.TileContext(nc) as tc:
        pid = nc.sync.partition_id()

        consts = tc.alloc_tile_pool(name="consts", bufs=1)
        usb = consts.tile([128, KCH, DO], FP16, tag="usb")
        wsb = consts.tile([128, JCH, KCH, 128], FP16, tag="wsb")
        id16 = consts.tile([128, 128], FP16, tag="id16")
        id32 = consts.tile([128, 128], FP32, tag="id32")
        bsb = consts.tile([128, JCH], FP32, tag="bsb")
        usb8 = consts.tile([128, KCH, DO], FP8, tag="usb8")
        warm = consts.tile([128, 512], FP16, tag="warm")
        zrow = consts.tile([1, D], FP32, tag="zrow")
        # HWDGE ring dispatch is ~630ns per DMA and serial, so only what
        # gates the first phase-1 matmul goes first: wsb z-plane + bias.
        # Everything else is dispatched after the tb=0 code (see below).
        nc.sync.dma_start(wsb[:, 0 : JCH // 2], wp[:, 0 : JCH // 2])
        nc.sync.dma_start(bsb[:], bp[:])

        def _deferred_const_dmas():
            nc.sync.dma_start(
                usb8[:], up8[:].rearrange("p (k m) -> p k m", k=KCH)
            )
            nc.sync.dma_start(usb[:], up[:].rearrange("p (k m) -> p k m", k=KCH))
            nc.sync.dma_start(id16[:], i16[:])
            nc.sync.dma_start(id32[:], i32[:])
            nc.vector.memset(zrow[:], 0.0)
            nc.sync.dma_start(gath9[0:1, :], zrow[:])

        az2 = tc.alloc_tile_pool(name="az2", bufs=1)
        azb = az2.tile([128, JCH, BC], FP16, tag="azb")

        # ------- fused phase 1 + pass 0, then zero-carry Picard passes -----
        with (
            tc.tile_pool(name="p1", bufs=3) as p1,
            tc.tile_pool(name="p0g", bufs=12) as p0g,
            tc.tile_pool(name="st", bufs=1) as st,
            tc.tile_pool(name="zh", bufs=4) as zh,
            tc.tile_pool(name="dd", bufs=3) as dd,
            tc.tile_pool(name="carry", bufs=2) as carryp,
            tc.tile_pool(name="outs", bufs=4) as outs,
            tc.tile_pool(name="fix", bufs=1) as fix,
            tc.tile_pool(name="fzh", bufs=8) as fzh,
            tc.tile_pool(name="ps2", bufs=(psbufs or 6), space="PSUM") as ps2,
            tc.tile_pool(name="pst2", bufs=2, space="PSUM") as pst2,
        ):
            # dependency-free warm-up matmuls ramp the PE p-state during
            # the startup DMA window so the first real matmuls run full-speed
            if nwarm:
                nc.vector.memset(warm[:], 0.0)
                for _ in range(nwarm):
                    pw = ps2.tile([128, 512], FP32, tag="psg")
                    nc.tensor.matmul(
                        pw[:], warm[:, 0:128], warm[:], start=True, stop=True
                    )
            mx = st.tile([128, KCH, BC + 1], FP16, tag="mx")
            m8 = st.tile([128, KCH, BC + 1], FP8, tag="m8")
            m32 = st.tile([128, KCH, BC], FP16, tag="m32")
            zcar = st.tile([128, KCH], FP32, tag="zcar")
            nc.vector.memset(zcar[:], 0.0)
            for c in range(KCH):
                nc.vector.tensor_copy(mx[:, c, 0:1], zcar[:, c : c + 1])
                nc.vector.memset(m8[:, c, 0:1], 0.0)

            # phase 1 computes psa = x.W in PSUM; pass 0's gates are a second
            # activation straight off the same psum (no fp16 round trip, no
            # PE re-injection), with d0/d1/scan quarters chained per tb
            for tb in range(BC // 512):
                if tb == 1:
                    _deferred_const_dmas()
                xT = p1.tile([128, KCH, 512], FP16, tag="xT")
                # ACT ring: x transfers run parallel to the weight DMAs on SP
                nc.scalar.dma_start(xT[:], xt[:, tb, :, :])
                if tb == 0:
                    # h-plane of W: after tb0's xT so it doesn't gate the
                    # first matmul, but before the j-loop that reads it
                    nc.sync.dma_start(
                        wsb[:, JCH // 2 : JCH], wp[:, JCH // 2 : JCH]
                    )
                # z/h pairs adjacent: chunk c's gate pair completes after 2
                # acts, and its d0/d1 + scan quarter + fp8 cast issue right
                # away, shortening the tail chain into pass 1
                for c in range(KCH):
                    gq = {}
                    for j in (c, c + KCH):
                        psa = ps2.tile([128, 512], FP32, tag="psg")
                        for k in range(KCH):
                            nc.tensor.matmul(
                                psa[:],
                                wsb[:, j, k, :],
                                xT[:, k, :],
                                start=(k == 0),
                                stop=(k == KCH - 1),
                            )
                        if j % 2 == 0:
                            nc.scalar.activation(
                                azb[:, j, tb * 512 : (tb + 1) * 512],
                                psa[:],
                                AF.Identity,
                                bias=bsb[:, j : j + 1],
                            )
                        else:
                            # balance the fused phase-1 ACT stream: odd-j az
                            # casts (psum + bias -> fp16) ride on DVE
                            nc.vector.tensor_scalar_add(
                                azb[:, j, tb * 512 : (tb + 1) * 512],
                                psa[:],
                                bsb[:, j : j + 1],
                            )
                        g = p0g.tile([128, 512], FP16, tag="g0")
                        nc.scalar.activation(
                            g[:],
                            psa[:],
                            AF.Sigmoid if j < KCH else AF.Tanh,
                            bias=bsb[:, j : j + 1],
                        )
                        gq[j] = g
                    d0q = p0g.tile([128, 512], FP16, tag="g0")
                    d1q = p0g.tile([128, 512], FP16, tag="g0")
                    nc.vector.tensor_scalar(
                        d0q[:], gq[c][:], -1.0, 1.0, ALU.mult, ALU.add
                    )
                    nc.vector.tensor_mul(d1q[:], gq[c][:], gq[c + KCH][:])
                    init = (
                        zcar[:, c : c + 1]
                        if tb == 0
                        else m8[:, c, tb * 512 : tb * 512 + 1]
                    )
                    nc.vector.tensor_tensor_scan(
                        m8[:, c, 1 + tb * 512 : 1 + (tb + 1) * 512],
                        d0q[:],
                        d1q[:],
                        init,
                        ALU.mult,
                        ALU.add,
                    )

            fcar = None
            for p in range(1, npass):
                first = p == 0
                last = p == npass - 1
                for c in range(KCH):
                    zt = zh.tile([128, BC], FP16, tag="zt")
                    ht = zh.tile([128, BC], FP16, tag="ht")
                    d0 = dd.tile([128, BC], FP16, tag="d0")
                    d1 = dd.tile([128, BC], FP16, tag="d1")
                    nh = 1
                    for sp in range(NSUB // nh):  # psum macro groups
                        for j, dst, fn in (
                            (c, zt, AF.Sigmoid),
                            (c + KCH, ht, AF.Tanh),
                        ):
                            psg = ps2.tile([128, 512], FP32, tag="psg")
                            fp8p = (not first) and p <= n8
                            for h in range(nh):
                                s = nh * sp + h
                                hsl = slice(h * 512, (h + 1) * 512)
                                nc.tensor.matmul(
                                    psg[:, hsl],
                                    id16[:],
                                    azb[:, j, s * 512 : (s + 1) * 512],
                                    start=True,
                                    stop=first,
                                )
                                if fp8p:
                                    # fp8e4 DoubleRow: 2 k-chunks per matmul
                                    # at 0.5 cycles/row (tile_matmul pattern:
                                    # lhsT [128,2,128], rhs [128,2,512])
                                    for kp in range(2):
                                        nc.tensor.matmul(
                                            psg[:, hsl],
                                            usb8[
                                                :,
                                                2 * kp : 2 * kp + 2,
                                                j * 128 : (j + 1) * 128,
                                            ],
                                            m8[
                                                :,
                                                2 * kp : 2 * kp + 2,
                                                s * 512 : s * 512 + 512,
                                            ],
                                            start=False,
                                            stop=(kp == 1),
                                            perf_mode=mybir.MatmulPerfMode.DoubleRow,
                                            skip_group_check=True,
                                        )
                                elif not first:
                                    korder = (
                                        [(c + i) % KCH for i in range(KCH)]
                                        if korder_rot
                                        else list(range(KCH))
                                    )
                                    for ki, k in enumerate(korder):
                                        nc.tensor.matmul(
                                            psg[:, hsl],
                                            usb[:, k, j * 128 : (j + 1) * 128],
                                            mx[:, k, s * 512 : s * 512 + 512],
                                            start=False,
                                            stop=(ki == KCH - 1),
                                        )
                            nc.scalar.activation(
                                dst[:, sp * 512 * nh : (sp + 1) * 512 * nh],
                                psg[:],
                                fn,
                            )
                        # subtile-granular d0/d1 + chained scan: the next
                        # pass's matmuls wait only on the first scan quarter
                        for h in range(nh):
                            s = nh * sp + h
                            sl = slice(s * 512, (s + 1) * 512)
                            nc.vector.tensor_scalar(
                                d0[:, sl], zt[:, sl], -1.0, 1.0, ALU.mult, ALU.add
                            )
                            nc.vector.tensor_mul(d1[:, sl], zt[:, sl], ht[:, sl])
                            # passes feeding an fp8 pass scan straight to
                            # the fp8 shadow state (fp32 scan accumulator;
                            # mx is only refreshed by pass n8 for the fp16
                            # passes) -- drops the Pool cast chain hop
                            to8 = p < n8
                            if s == 0:
                                init = zcar[:, c : c + 1]
                            elif last:
                                init = m32[:, c, s * 512 - 1 : s * 512]
                            elif to8:
                                init = m8[:, c, s * 512 : s * 512 + 1]
                            else:
                                init = mx[:, c, s * 512 : s * 512 + 1]
                            if last:
                                out_ap = m32[:, c, sl]
                            elif to8:
                                out_ap = m8[:, c, 1 + s * 512 : 1 + (s + 1) * 512]
                            else:
                                out_ap = mx[:, c, 1 + s * 512 : 1 + (s + 1) * 512]
                            nc.vector.tensor_tensor_scan(
                                out_ap, d0[:, sl], d1[:, sl], init, ALU.mult, ALU.add
                            )

                if p == npass - 2:
                    # single exchange, overlapping the final pass: my current
                    # final state -> all cores; I keep the previous core's.
                    cout = carryp.tile([128, KCH], FP32, tag="cout")
                    for c in range(KCH):
                        nc.vector.tensor_copy(
                            cout[:, c : c + 1], mx[:, c, BC : BC + 1]
                        )
                    nc.sync.dma_start(
                        cin[:].rearrange("o (p c) -> o p c", c=KCH), cout[:]
                    )
                    if not sim_nocc:
                        nc.gpsimd.collective_compute(
                            "AllGather",
                            ALU.bypass,
                            replica_groups=[list(range(NCORE))],
                            ins=[cin[:]],
                            outs=[gath9[1 : NCORE + 1, :]],
                        )
                    fcar = carryp.tile([128, KCH], FP32, tag="fcar")
                    nc.sync.dma_start(
                        fcar[:],
                        gath9[ds(pid, 1), :].rearrange("o (p c) -> o p c", c=KCH),
                    )

            def emit_out_group(tt):
                # fp16 transposes run at 1 cycle/row (fp32 needs 2); the
                # fp32 upcast happens in the DVE copy below
                yst = outs.tile([128, D], FP32, tag="yst")
                for c in range(KCH):
                    psy = pst2.tile([128, 128], FP16, tag="psy")
                    nc.tensor.transpose(
                        psy[:], m32[:, c, tt * 128 : (tt + 1) * 128], id16[:]
                    )
                    nc.vector.tensor_copy(yst[:, c * 128 : (c + 1) * 128], psy[:])
                nc.sync.dma_start(ys[tt * 128 : (tt + 1) * 128, :], yst[:])

            # ---------------- head-window fixup with the true carry --------
            # chunk-Jacobi with double-buffered state so all 4 feature chunks
            # pipeline per pass; tail-output groups (rows >= fixw are final
            # already) are emitted between passes so PE fills latency stalls
            nhead = max(1, fixw // 128)
            ngrp = BC // 128 - nhead
            mwA = fix.tile([128, KCH, fixw + 1], FP16, tag="mwA")
            mwB = fix.tile([128, KCH, fixw + 1], FP16, tag="mwB")
            for mwt in (mwA, mwB):
                for c in range(KCH):
                    nc.vector.tensor_copy(mwt[:, c, 0:1], fcar[:, c : c + 1])
            for c in range(KCH):
                nc.vector.tensor_copy(mwA[:, c, 1:fixw], m32[:, c, 0 : fixw - 1])
            emitted = 0
            for f in range(fixp):
                lastf = f == fixp - 1
                if fix_jacobi:
                    mwr = mwA if f % 2 == 0 else mwB
                    mww = mwB if f % 2 == 0 else mwA
                else:
                    mwr = mww = mwA
                # all four chunks of a gate type share one psum tile as
                # 64-col windows -> ONE 256-wide activation per gate type
                # (the small per-chunk acts were overhead-dominated)
                zw4 = fzh.tile([128, KCH, fixw], FP16, tag="w4")
                hw4 = fzh.tile([128, KCH, fixw], FP16, tag="w4")
                for base, dst, fn in ((0, zw4, AF.Sigmoid), (KCH, hw4, AF.Tanh)):
                    psw = ps2.tile([128, 512], FP32, tag="psg")
                    for c in range(KCH):
                        j = base + c
                        wsl = slice(c * fixw, (c + 1) * fixw)
                        nc.tensor.matmul(
                            psw[:, wsl],
                            id16[:],
                            azb[:, j, 0:fixw],
                            start=True,
                            stop=False,
                        )
                        for k in range(KCH):
                            nc.tensor.matmul(
                                psw[:, wsl],
                                usb[:, k, j * 128 : (j + 1) * 128],
                                mwr[:, k, 0:fixw],
                                start=False,
                                stop=(k == KCH - 1),
                            )
                    nc.scalar.activation(
                        dst[:], psw[:, 0 : KCH * fixw], fn
                    )
                d0w4 = fzh.tile([128, KCH, fixw], FP16, tag="w4")
                d1w4 = fzh.tile([128, KCH, fixw], FP16, tag="w4")
                nc.vector.tensor_scalar(
                    d0w4[:], zw4[:], -1.0, 1.0, ALU.mult, ALU.add
                )
                nc.vector.tensor_mul(d1w4[:], zw4[:], hw4[:])
                for c in range(KCH):
                    out_ap = m32[:, c, 0:fixw] if lastf else mww[:, c, 1 : fixw + 1]
                    nc.vector.tensor_tensor_scan(
                        out_ap,
                        d0w4[:, c, :],
                        d1w4[:, c, :],
                        fcar[:, c : c + 1],
                        ALU.mult,
                        ALU.add,
                    )
                ntail = (ngrp * (f + 1)) // fixp
                while emitted < ntail:
                    emit_out_group(nhead + emitted)
                    emitted += 1

            # head output (depends on the fixup-written rows)
            for tt in range(nhead):
                emit_out_group(tt)

        az2.release()
        consts.release()

    _split_multi_waits(nc)
    return nc



_CACHE = {}


def _make_runner(nc):
    """Single-core PJRT runner with a persistent jit cache (run_bass_via_pjrt
    builds a fresh closure per call, forcing a full recompile; this keeps the
    jitted body alive so repeat calls only pay transfer + execute)."""
    import jax
    from concourse import bass2jax

    bass2jax.install_neuronx_cc_hook()
    part_name = nc.partition_id_tensor.name if nc.partition_id_tensor else None
    in_names, out_names, out_avals = [], [], []
    for alloc in nc.m.functions[0].allocations:
        if not isinstance(alloc, mybir.MemoryLocationSet):
            continue
        name = alloc.memorylocations[0].name
        if alloc.kind == "ExternalInput":
            if name != part_name:
                in_names.append(name)
        elif alloc.kind == "ExternalOutput":
            out_names.append(name)
            out_avals.append(
                jax.core.ShapedArray(
                    tuple(alloc.tensor_shape), mybir.dt.np(alloc.dtype)
                )
            )
    n_params = len(in_names)
    all_names = in_names + out_names
    if part_name is not None:
        all_names = all_names + [part_name]
    all_names = tuple(all_names)
    donate = tuple(range(n_params, n_params + len(out_names)))

    def _body(*args):
        operands = list(args)
        if part_name is not None:
            operands.append(bass2jax.partition_id_tensor())
        outs = bass2jax._bass_exec_p.bind(
            *operands,
            out_avals=tuple(out_avals),
            in_names=all_names,
            out_names=tuple(out_names),
            lowering_input_output_aliases=(),
            sim_require_finite=True,
            sim_require_nnan=True,
            nc=nc,
        )
        return tuple(outs)

    jitted = jax.jit(_body, donate_argnums=donate, keep_unused=True)

    def run(in_map):
        args = [np.asarray(in_map[n]) for n in in_names[:n_params]]
        args += [np.zeros(a.shape, a.dtype) for a in out_avals]
        outs = jax.block_until_ready(jitted(*args))
        return {name: outs[i] for i, name in enumerate(out_names)}

    return run


def _host_prep(inputs):
    import ml_dtypes

    wp = np.concatenate(
        [np.asarray(inputs["Wz"], np.float32), np.asarray(inputs["Wh"], np.float32)],
        axis=1,
    ).astype(np.float16)
    up32 = np.concatenate(
        [np.asarray(inputs["Uz"], np.float32), np.asarray(inputs["Uh"], np.float32)],
        axis=1,
    )
    up = up32.astype(np.float16)
    up8 = up32.astype(ml_dtypes.float8_e4m3)

    def pack_pm(w):  # [D, DO] -> [128, KCH*DO] partition-major
        return np.ascontiguousarray(
            w.reshape(KCH, 128, DO).transpose(1, 0, 2).reshape(128, KCH * DO)
        )

    # wp j-chunked for streaming: [128, j, k, 128]
    wp = np.ascontiguousarray(
        wp.reshape(KCH, 128, JCH, 128).transpose(1, 2, 0, 3)
    )
    up = pack_pm(up)
    up8 = pack_pm(up8)
    bpack = (
        np.concatenate(
            [np.asarray(inputs["bz"], np.float32), np.asarray(inputs["bh"], np.float32)]
        )
        .reshape(JCH, 128)
        .T.copy()
        .astype(np.float32)
    )
    return {
        "wp": wp,
        "up": up,
        "up8": up8,
        "bp": bpack,
        "i16": np.eye(128, dtype=np.float16),
        "i32": np.eye(128, dtype=np.float32),
    }


def kernel(**inputs: np.ndarray) -> np.ndarray:
    """8-core block-Jacobi fixed point (default). Set MEMORY_KERNEL_MODE=single
    to fall back to the single-core blockwise kernel."""
    import os

    import jax

    x = np.ascontiguousarray(inputs["x"], dtype=np.float32)
    common = _host_prep(inputs)
    # Pin a real neuron device: with a CPU default device the bass_exec
    # primitive lowers to the MultiCoreSim fallback instead of hardware.
    dev = [d for d in jax.devices() if d.platform != "cpu"][0]

    single = os.environ.get("MEMORY_KERNEL_MODE", "").lower() == "single"
    last_exc = None
    for attempt in range(3):
        try:
            if single:
                if "nc1" not in _CACHE:
                    _CACHE["nc1"] = build_kernel()
                    _CACHE["runner1"] = _make_runner(_CACHE["nc1"])
                with jax.default_device(dev):
                    out = _CACHE["runner1"]({"x": x, **common})
                return np.ascontiguousarray(out["ys"])
            if "nc8" not in _CACHE:
                _CACHE["nc8"] = build_kernel8()
            xt_all = x.T.astype(np.float16)  # [D, T]
            ntb = BC // 512
            in_maps = []
            for c in range(NCORE):
                xc = xt_all[:, c * BC : (c + 1) * BC]  # [D, BC]
                # -> [128, tb, k, 512] partition-major
                xp = np.ascontiguousarray(
                    xc.reshape(KCH, 128, ntb, 512).transpose(1, 2, 0, 3)
                )
                in_maps.append({"xt": xp, **common})
            with jax.default_device(dev):
                res = run_bass_kernel_spmd(
                    _CACHE["nc8"], in_maps, core_ids=list(range(NCORE))
                )
            out = np.concatenate(
                [np.asarray(res.results[c]["ys"]) for c in range(NCORE)], axis=0
            )
            # transient first-exec glitches can yield garbage; |m_t| <= 1 by
            # construction (convex combination of tanh values), so re-run on
            # any non-finite or out-of-range output
            if not np.isfinite(out).all() or np.abs(out).max() > 1.5:
                raise RuntimeError("NRT transient: non-finite kernel output")
            return out
        except Exception as e:  # transient NRT device errors on first exec
            last_exc = e
            if (
                "UNRECOVERABLE" not in str(e)
                and "NRT" not in str(e)
                and "non-finite" not in str(e)
            ):
                raise
    raise last_exc


if __name__ == "__main__":
    rng = np.random.RandomState(0)
    ins = {
        "x": rng.randn(T, D).astype(np.float32),
        "Wz": (rng.randn(D, D) / np.sqrt(D)).astype(np.float32),
        "Uz": (rng.randn(D, D) / np.sqrt(D)).astype(np.float32),
        "bz": np.zeros(D, np.float32),
        "Wh": (rng.randn(D, D) / np.sqrt(D)).astype(np.float32),
        "Uh": (rng.randn(D, D) / np.sqrt(D)).astype(np.float32),
        "bh": np.zeros(D, np.float32),
    }
    out = kernel(**ins)
    print("out", out.shape, out.dtype, np.abs(out).max())



# revision 73
# speedup vs baseline: 1.1219x; 1.1219x over previous
"""Trainium2 Bass kernel for nn_Memory (GRU-style scan over 16384 rows, d=512).

Strategy: the recurrence m_t = (1-z_t) m_{t-1} + z_t h_t is *linear in m given
the gates*, and the gates depend on m_{t-1} through two 512x512 matvecs. Each
block is solved by Picard iteration: compute all gates from the previous
iterate's shifted states with large batched matmuls (full PE utilization),
then re-propagate the states exactly with the hardware linear scan primitive
(tensor_tensor_scan). In-pass feature-chunk ordering makes this Gauss-Seidel,
converging at ~0.5-0.6x error per pass.

Default (8-core single-exchange scheme, build_kernel8): each core owns a
2048-row block. The GRU dynamics forget the block-initial state within ~48
rows (numpy: state diff 4e-6 by t=48), so each block's FINAL state is
independent of its incoming carry. Each core therefore converges its block
locally with a zero carry (the az-only init pass is fused into phase 1 as a
second activation off the same x.W psum, then 4 passes with fp8e4 DoubleRow
U-matmuls at 0.5 cycles/row and 4 fp16 passes), ONE tiny AllGather
(issued after pass 7, overlapping the final pass) distributes the block-final
states, and only the first 64 rows are re-solved with the true carry (10
chunk-Jacobi window passes, double-buffered so all 4 feature chunks pipeline)
while the tail rows stream out between window passes. Measured on hw:
rel L2 ~2.5e-3, max rel ~8.8e-3 (tolerance 2e-2).

Per core and per full pass: az injection via fp16 identity matmul + U-matvec
accumulation against stationary [Uz|Uh] tiles; sigmoid/tanh on ACT straight
out of PSUM; d0=1-z, d1=z*h and the chained 512-wide scans on DVE (fp32 scan
accumulator; passes feeding a DoubleRow pass scan straight to the fp8 shadow
state, pass 4 refreshes the fp16 state for the fp16 passes, and the final
pass + output store fp16 for 1-cycle/row transposes).

Host-side prep (free, like the weight packing): x is transposed/cast to fp16
partition-major [128, tb, k, 512] so phase 1 needs no PE transposes, and
W/U/U8 are packed partition-major so every weight DMA is one contiguous run
per partition (HWDGE dispatch is ~630ns per DMA and serial at startup; only
the W z-plane + bias gate the first matmul).

MEMORY_KERNEL_MODE=single selects the legacy single-core variant
(build_kernel); it predates the packed host layouts and is retained for
reference only.
"""

import sys

sys.path.insert(0, "/opt/trn_rl_repo")

import numpy as np

import concourse.bass as bass
import concourse.mybir as mybir
import concourse.tile as tile
from concourse.bass import ds
from concourse.bass_utils import run_bass_kernel_spmd

T = 16384
D = 512  # in/out features
DO = 2 * D  # packed gate outputs (z | h)
B = 2048  # fixed-point block length
NBLK = T // B
NPASS = 17  # gate/scan passes per block (pass 0: no U-matmul; last: fp32 scan)
KCH = D // 128  # 4 contraction chunks
JCH = DO // 128  # 8 output chunks (0..3 -> z, 4..7 -> h)
NSUB = B // 512  # 512-column matmul subtiles per block

FP32 = mybir.dt.float32
FP16 = mybir.dt.float16
FP8 = mybir.dt.float8e4
AF = mybir.ActivationFunctionType
ALU = mybir.AluOpType


def _apply_tile_drain_patch():
    """This container's walrus rejects >1 sync-wait on the TileContext exit
    Drain (setupSyncWait/CTRL_NO_STRUCT). Split the accumulated end-of-kernel
    waits into one Drain per semaphore."""
    import bass_rust

    def _drain_and_barrier(self, tick_clock, wait_clock):
        drain_inst = self.nc.sync.drain()
        wait_clock.add_sem_waits(
            drain_inst.ins, tile.ScopedClock({None: tick_clock.global_clock})
        )
        si = drain_inst.ins.sync_info
        if si is not None and len(si.on_wait) > 1:
            waits = list(si.on_wait)
            si.on_wait = waits[:1]
            for w in waits[1:]:
                d2 = self.nc.sync.drain()
                s2 = d2.ins.sync_info
                if s2 is None:
                    d2.ins.sync_info = bass_rust.SyncInfo(on_wait=[w], on_update=[])
                else:
                    s2.on_wait = [w]
        self.nc.all_engine_barrier()
        assert self.sems is not None
        popped = self.nc._tile_sem_poison_stack.pop()
        assert popped is self._sem_poison
        self.nc.clear_and_free_semaphores(list(self.sems.allocated().values()))
        self.nc.all_engine_barrier()

    tile.TileContext._drain_and_barrier = _drain_and_barrier


def _split_multi_waits(nc):
    """This walrus build encodes at most ONE sync-wait per hardware
    instruction. Hoist extra waits onto same-engine NoOps placed immediately
    before the owning instruction (engines execute block order, so the waits
    still all complete before it runs)."""
    import bass_rust

    nid = 0
    for f in nc.m.functions:
        for b in f.blocks:
            out = []
            changed = False
            for ins in b.instructions:
                si = ins.sync_info
                if si is not None and len(si.on_wait) > 1:
                    waits = list(si.on_wait)
                    for w in waits[:-1]:
                        nop = mybir.InstNoOp(name=f"I-waitsplit-{nid}", ins=[], outs=[])
                        nid += 1
                        nop.engine = ins.engine
                        nop.sync_info = bass_rust.SyncInfo(on_wait=[w], on_update=[])
                        out.append(nop)
                    si.on_wait = waits[-1:]
                    changed = True
                out.append(ins)
            if changed:
                b.instructions = out


def build_kernel(npass=NPASS, phase1=True, fixpoint=True):
    _apply_tile_drain_patch()
    nc = bass.Bass("TRN2")

    x = nc.dram_tensor("x", [T, D], FP32, kind="ExternalInput")
    wp = nc.dram_tensor("wp", [D, DO], FP16, kind="ExternalInput")  # [Wz|Wh]
    up = nc.dram_tensor("up", [D, DO], FP16, kind="ExternalInput")  # [Uz|Uh]
    i16 = nc.dram_tensor("i16", [128, 128], FP16, kind="ExternalInput")
    i32 = nc.dram_tensor("i32", [128, 128], FP32, kind="ExternalInput")
    bp = nc.dram_tensor("bp", [128, JCH], FP32, kind="ExternalInput")  # bias chunks
    ys = nc.dram_tensor("ys", [T, D], FP32, kind="ExternalOutput")

    with tile.TileContext(nc) as tc:
        consts = tc.alloc_tile_pool(name="consts", bufs=1)
        usb = consts.tile([128, KCH, DO], FP16, tag="usb")
        wsb = consts.tile([128, JCH, KCH, 128], FP16, tag="wsb")
        id16 = consts.tile([128, 128], FP16, tag="id16")
        id32 = consts.tile([128, 128], FP32, tag="id32")
        bsb = consts.tile([128, JCH], FP32, tag="bsb")
        nc.sync.dma_start(usb[:], up.rearrange("(k p) m -> p k m", p=128))
        nc.sync.dma_start(wsb[:], wp.rearrange("(k p) m -> p k m", p=128))
        nc.sync.dma_start(id16[:], i16[:])
        nc.sync.dma_start(id32[:], i32[:])
        nc.sync.dma_start(bsb[:], bp[:])

        dram = tc.alloc_tile_pool(name="dram", bufs=1, space="DRAM")
        # AZ^T/AH^T staged as [feat%128, out-chunk j, t]; j<4: z, j>=4: h
        azt = dram.tile([128, JCH, T], FP16, tag="azt")

        # ---------------- phase 1: x^T and AZ/AH ----------------
        with (
            tc.tile_pool(name="p1", bufs=3) as p1,
            tc.tile_pool(name="p1ps", bufs=4, space="PSUM") as p1ps,
            tc.tile_pool(name="p1az", bufs=2, space="PSUM") as p1az,
        ):
            for tb in range(T // 512 if phase1 else 0):
                xT = p1.tile([128, KCH, 512], FP16, tag="xT")
                for s in range(4):
                    xt = p1.tile([128, D], FP32, tag="xt")
                    t0 = tb * 512 + s * 128
                    nc.sync.dma_start(xt[:], x[t0 : t0 + 128, :])
                    for k in range(KCH):
                        pst = p1ps.tile([128, 128], FP32, tag="pst")
                        nc.tensor.transpose(
                            pst[:], xt[:, k * 128 : (k + 1) * 128], id32[:]
                        )
                        nc.vector.tensor_copy(
                            xT[:, k, s * 128 : (s + 1) * 128], pst[:]
                        )
                az16 = p1.tile([128, JCH, 512], FP16, tag="az16")
                for j in range(JCH):
                    psa = p1az.tile([128, 512], FP32, tag="psa")
                    for k in range(KCH):
                        nc.tensor.matmul(
                            psa[:],
                            wsb[:, j, k, :],
                            xT[:, k, :],
                            start=(k == 0),
                            stop=(k == KCH - 1),
                        )
                    # az16 = psum + bias_chunk (per-partition), cast fp16
                    nc.scalar.activation(
                        az16[:, j, :], psa[:], AF.Identity, bias=bsb[:, j : j + 1]
                    )
                nc.sync.dma_start(azt[:, :, tb * 512 : (tb + 1) * 512], az16[:])

        # ---------------- phase 2: blockwise fixed point ----------------
        with (
            tc.tile_pool(name="st", bufs=1) as st,
            tc.tile_pool(name="gates", bufs=1) as gates,
            tc.tile_pool(name="az2", bufs=1) as az2,
            tc.tile_pool(name="carry", bufs=2) as carryp,
            tc.tile_pool(name="outs", bufs=4) as outs,
            tc.tile_pool(name="ps2", bufs=6, space="PSUM") as ps2,
            tc.tile_pool(name="pst2", bufs=2, space="PSUM") as pst2,
        ):
            carry = carryp.tile([128, KCH], FP32, tag="carry")
            nc.vector.memset(carry[:], 0.0)

            for b in range(NBLK if fixpoint else 0):
                azb = az2.tile([128, JCH, B], FP16, tag="azb")
                nc.sync.dma_start(azb[:], azt[:, :, b * B : (b + 1) * B])

                # states, shifted by one: col 0 = carry, cols 1.. = m_t
                mx = st.tile([128, KCH, B + 1], FP16, tag="mx")
                m32 = st.tile([128, KCH, B], FP32, tag="m32")
                for c in range(KCH):
                    nc.vector.tensor_copy(mx[:, c, 0:1], carry[:, c : c + 1])

                zt = gates.tile([128, KCH, B], FP16, tag="zt")
                ht = gates.tile([128, KCH, B], FP16, tag="ht")
                d0 = gates.tile([128, KCH, B], FP16, tag="d0")
                d1 = gates.tile([128, KCH, B], FP16, tag="d1")

                for p in range(npass):
                    first = p == 0
                    last = p == npass - 1
                    for c in range(KCH):
                        for j in (c, c + KCH):  # z-chunk then h-chunk
                            for s in range(NSUB):
                                psg = ps2.tile([128, 512], FP32, tag="psg")
                                nc.tensor.matmul(
                                    psg[:],
                                    id16[:],
                                    azb[:, j, s * 512 : (s + 1) * 512],
                                    start=True,
                                    stop=first,
                                )
                                if not first:
                                    for k in range(KCH):
                                        nc.tensor.matmul(
                                            psg[:],
                                            usb[:, k, j * 128 : (j + 1) * 128],
                                            mx[:, k, s * 512 : s * 512 + 512],
                                            start=False,
                                            stop=(k == KCH - 1),
                                        )
                                dst = zt if j < KCH else ht
                                fn = AF.Sigmoid if j < KCH else AF.Tanh
                                nc.scalar.activation(
                                    dst[:, c, s * 512 : (s + 1) * 512], psg[:], fn
                                )
                        # d0 = 1 - z ; d1 = z * h
                        nc.vector.tensor_scalar(
                            d0[:, c, :], zt[:, c, :], -1.0, 1.0, ALU.mult, ALU.add
                        )
                        nc.vector.tensor_mul(d1[:, c, :], zt[:, c, :], ht[:, c, :])
                        # m_t = d0_t * m_{t-1} + d1_t  (exact sequential scan)
                        out_ap = m32[:, c, :] if last else mx[:, c, 1 : B + 1]
                        nc.vector.tensor_tensor_scan(
                            out_ap,
                            d0[:, c, :],
                            d1[:, c, :],
                            carry[:, c : c + 1],
                            ALU.mult,
                            ALU.add,
                        )

                ncarry = carryp.tile([128, KCH], FP32, tag="carry")
                for c in range(KCH):
                    nc.vector.tensor_copy(ncarry[:, c : c + 1], m32[:, c, B - 1 : B])
                carry = ncarry

                # transpose states back to [t, feat] rows and store
                for tt in range(B // 128):
                    yst = outs.tile([128, D], FP32, tag="yst")
                    for c in range(KCH):
                        psy = pst2.tile([128, 128], FP32, tag="psy")
                        nc.tensor.transpose(
                            psy[:], m32[:, c, tt * 128 : (tt + 1) * 128], id32[:]
                        )
                        nc.vector.tensor_copy(
                            yst[:, c * 128 : (c + 1) * 128], psy[:]
                        )
                    t0 = b * B + tt * 128
                    nc.sync.dma_start(ys[t0 : t0 + 128, :], yst[:])

        consts.release()
        dram.release()

    _split_multi_waits(nc)
    return nc



NCORE = 8
BC = T // NCORE  # rows per core in the 8-core kernel
NPASS8 = 8  # full-block passes (pass 0: az-only init + 7 Picard sweeps)
N8 = 4  # of which: passes 1..N8 use fp8e4 DoubleRow U-matmuls (4x PE rate)
NWARM = 14  # startup PE p-state warm-up matmuls (dependency-free)
NSUB8 = BC // 512
FIXW = 64  # head-window length re-solved after the single carry exchange
FIXP = 10  # fixup passes on the head window (chunk-Jacobi, double-buffered)


def build_kernel8(npass=NPASS8, sim_nocc=False, fixw=FIXW, fixp=FIXP,
                  wide_act=False, korder_rot=False, fix_jacobi=True,
                  psbufs=None, n8=N8, nwarm=NWARM):
    # sim_nocc: drop the AllGather (TimelineSim can't model collectives) so the
    # per-core occupancy can be cost-model-simulated; numerics become wrong.
    #
    # Single-exchange scheme: the GRU state forgets its block-initial carry
    # within ~48 steps (state diff 4e-6 by t=48), so each core's final state is
    # independent of its incoming carry. All full passes therefore run with a
    # zero carry; ONE AllGather (issued after pass npass-2, overlapping the
    # last pass) distributes the block-final states, and only the first `fixw`
    # rows are re-solved with the true carry (`fixp` cheap window passes).
    _apply_tile_drain_patch()
    nc = bass.Bass("TRN2", num_devices=NCORE)

    # host-packed partition-major layouts: one contiguous run per partition
    # keeps DMA descriptor counts (and SEQ dispatch time) minimal
    xt = nc.dram_tensor("xt", [128, BC // 512, KCH, 512], FP16, kind="ExternalInput")
    up8 = nc.dram_tensor("up8", [128, KCH * DO], FP8, kind="ExternalInput")
    wp = nc.dram_tensor("wp", [128, JCH, KCH, 128], FP16, kind="ExternalInput")
    up = nc.dram_tensor("up", [128, KCH * DO], FP16, kind="ExternalInput")
    i16 = nc.dram_tensor("i16", [128, 128], FP16, kind="ExternalInput")
    i32 = nc.dram_tensor("i32", [128, 128], FP32, kind="ExternalInput")
    bp = nc.dram_tensor("bp", [128, JCH], FP32, kind="ExternalInput")
    ys = nc.dram_tensor("ys", [BC, D], FP32, kind="ExternalOutput")

    cin = nc.dram_tensor("cin", [1, D], FP32)
    gath9 = nc.dram_tensor("gath9", [NCORE + 1, D], FP32, addr_space="Shared")

    with tile.TileContext(nc) as tc:
        pid = nc.sync.partition_id()

        consts = tc.alloc_tile_pool(name="consts", bufs=1)
        usb = consts.tile([128, KCH, DO], FP16, tag="usb")
        wsb = consts.tile([128, JCH, KCH, 128], FP16, tag="wsb")
        id16 = consts.tile([128, 128], FP16, tag="id16")
        id32 = consts.tile([128, 128], FP32, tag="id32")
        bsb = consts.tile([128, JCH], FP32, tag="bsb")
        usb8 = consts.tile([128, KCH, DO], FP8, tag="usb8")
        warm = consts.tile([128, 512], FP16, tag="warm")
        zrow = consts.tile([1, D], FP32, tag="zrow")
        # HWDGE ring dispatch is ~630ns per DMA and serial, so only what
        # gates the first phase-1 matmul goes first: wsb z-plane + bias.
        # Everything else is dispatched after the tb=0 code (see below).
        nc.sync.dma_start(wsb[:, 0 : JCH // 2], wp[:, 0 : JCH // 2])
        nc.sync.dma_start(bsb[:], bp[:])

        def _deferred_const_dmas():
            nc.sync.dma_start(
                usb8[:], up8[:].rearrange("p (k m) -> p k m", k=KCH)
            )
            nc.sync.dma_start(usb[:], up[:].rearrange("p (k m) -> p k m", k=KCH))
            nc.sync.dma_start(id16[:], i16[:])
            nc.sync.dma_start(id32[:], i32[:])
            nc.vector.memset(zrow[:], 0.0)
            nc.sync.dma_start(gath9[0:1, :], zrow[:])

        az2 = tc.alloc_tile_pool(name="az2", bufs=1)
        azb = az2.tile([128, JCH, BC], FP16, tag="azb")

        # ------- fused phase 1 + pass 0, then zero-carry Picard passes -----
        with (
            tc.tile_pool(name="p1", bufs=3) as p1,
            tc.tile_pool(name="p0g", bufs=12) as p0g,
            tc.tile_pool(name="st", bufs=1) as st,
            tc.tile_pool(name="zh", bufs=4) as zh,
            tc.tile_pool(name="dd", bufs=3) as dd,
            tc.tile_pool(name="carry", bufs=2) as carryp,
            tc.tile_pool(name="outs", bufs=4) as outs,
            tc.tile_pool(name="fix", bufs=1) as fix,
            tc.tile_pool(name="fzh", bufs=8) as fzh,
            tc.tile_pool(name="ps2", bufs=(psbufs or 6), space="PSUM") as ps2,
            tc.tile_pool(name="pst2", bufs=2, space="PSUM") as pst2,
        ):
            # dependency-free warm-up matmuls ramp the PE p-state during
            # the startup DMA window so the first real matmuls run full-speed
            if nwarm:
                nc.vector.memset(warm[:], 0.0)
                for _ in range(nwarm):
                    pw = ps2.tile([128, 512], FP32, tag="psg")
                    nc.tensor.matmul(
                        pw[:], warm[:, 0:128], warm[:], start=True, stop=True
                    )
            mx = st.tile([128, KCH, BC + 1], FP16, tag="mx")
            m8 = st.tile([128, KCH, BC + 1], FP8, tag="m8")
            m32 = st.tile([128, KCH, BC], FP16, tag="m32")
            zcar = st.tile([128, KCH], FP32, tag="zcar")
            nc.vector.memset(zcar[:], 0.0)
            for c in range(KCH):
                nc.vector.tensor_copy(mx[:, c, 0:1], zcar[:, c : c + 1])
                nc.vector.memset(m8[:, c, 0:1], 0.0)

            # phase 1 computes psa = x.W in PSUM; pass 0's gates are a second
            # activation straight off the same psum (no fp16 round trip, no
            # PE re-injection), with d0/d1/scan quarters chained per tb
            for tb in range(BC // 512):
                if tb == 1:
                    _deferred_const_dmas()
                xT = p1.tile([128, KCH, 512], FP16, tag="xT")
                # ACT ring: x transfers run parallel to the weight DMAs on SP
                nc.scalar.dma_start(xT[:], xt[:, tb, :, :])
                if tb == 0:
                    # h-plane of W: after tb0's xT so it doesn't gate the
                    # first matmul, but before the j-loop that reads it
                    nc.sync.dma_start(
                        wsb[:, JCH // 2 : JCH], wp[:, JCH // 2 : JCH]
                    )
                # z/h pairs adjacent: chunk c's gate pair completes after 2
                # acts, and its d0/d1 + scan quarter + fp8 cast issue right
                # away, shortening the tail chain into pass 1
                for c in range(KCH):
                    gq = {}
                    for j in (c, c + KCH):
                        psa = ps2.tile([128, 512], FP32, tag="psg")
                        for k in range(KCH):
                            nc.tensor.matmul(
                                psa[:],
                                wsb[:, j, k, :],
                                xT[:, k, :],
                                start=(k == 0),
                                stop=(k == KCH - 1),
                            )
                        if j % 2 == 0:
                            nc.scalar.activation(
                                azb[:, j, tb * 512 : (tb + 1) * 512],
                                psa[:],
                                AF.Identity,
                                bias=bsb[:, j : j + 1],
                            )
                        else:
                            # balance the fused phase-1 ACT stream: odd-j az
                            # casts (psum + bias -> fp16) ride on DVE
                            nc.vector.tensor_scalar_add(
                                azb[:, j, tb * 512 : (tb + 1) * 512],
                                psa[:],
                                bsb[:, j : j + 1],
                            )
                        g = p0g.tile([128, 512], FP16, tag="g0")
                        nc.scalar.activation(
                            g[:],
                            psa[:],
                            AF.Sigmoid if j < KCH else AF.Tanh,
                            bias=bsb[:, j : j + 1],
                        )
                        gq[j] = g
                    d0q = p0g.tile([128, 512], FP16, tag="g0")
                    d1q = p0g.tile([128, 512], FP16, tag="g0")
                    nc.vector.tensor_scalar(
                        d0q[:], gq[c][:], -1.0, 1.0, ALU.mult, ALU.add
                    )
                    nc.vector.tensor_mul(d1q[:], gq[c][:], gq[c + KCH][:])
                    init = (
                        zcar[:, c : c + 1]
                        if tb == 0
                        else m8[:, c, tb * 512 : tb * 512 + 1]
                    )
                    nc.vector.tensor_tensor_scan(
                        m8[:, c, 1 + tb * 512 : 1 + (tb + 1) * 512],
                        d0q[:],
                        d1q[:],
                        init,
                        ALU.mult,
                        ALU.add,
                    )

            fcar = None
            for p in range(1, npass):
                first = p == 0
                last = p == npass - 1
                for c in range(KCH):
                    zt = zh.tile([128, BC], FP16, tag="zt")
                    ht = zh.tile([128, BC], FP16, tag="ht")
                    d0 = dd.tile([128, BC], FP16, tag="d0")
                    d1 = dd.tile([128, BC], FP16, tag="d1")
                    nh = 1
                    for sp in range(NSUB // nh):  # psum macro groups
                        for j, dst, fn in (
                            (c, zt, AF.Sigmoid),
                            (c + KCH, ht, AF.Tanh),
                        ):
                            psg = ps2.tile([128, 512], FP32, tag="psg")
                            fp8p = (not first) and p <= n8
                            for h in range(nh):
                                s = nh * sp + h
                                hsl = slice(h * 512, (h + 1) * 512)
                                nc.tensor.matmul(
                                    psg[:, hsl],
                                    id16[:],
                                    azb[:, j, s * 512 : (s + 1) * 512],
                                    start=True,
                                    stop=first,
                                )
                                if fp8p:
                                    # fp8e4 DoubleRow: 2 k-chunks per matmul
                                    # at 0.5 cycles/row (tile_matmul pattern:
                                    # lhsT [128,2,128], rhs [128,2,512])
                                    for kp in range(2):
                                        nc.tensor.matmul(
                                            psg[:, hsl],
                                            usb8[
                                                :,
                                                2 * kp : 2 * kp + 2,
                                                j * 128 : (j + 1) * 128,
                                            ],
                                            m8[
                                                :,
                                                2 * kp : 2 * kp + 2,
                                                s * 512 : s * 512 + 512,
                                            ],
                                            start=False,
                                            stop=(kp == 1),
                                            perf_mode=mybir.MatmulPerfMode.DoubleRow,
                                            skip_group_check=True,
                                        )
                                elif not first:
                                    korder = (
                                        [(c + i) % KCH for i in range(KCH)]
                                        if korder_rot
                                        else list(range(KCH))
                                    )
                                    for ki, k in enumerate(korder):
                                        nc.tensor.matmul(
                                            psg[:, hsl],
                                            usb[:, k, j * 128 : (j + 1) * 128],
                                            mx[:, k, s * 512 : s * 512 + 512],
                                            start=False,
                                            stop=(ki == KCH - 1),
                                        )
                            nc.scalar.activation(
                                dst[:, sp * 512 * nh : (sp + 1) * 512 * nh],
                                psg[:],
                                fn,
                            )
                        # subtile-granular d0/d1 + chained scan: the next
                        # pass's matmuls wait only on the first scan quarter
                        for h in range(nh):
                            s = nh * sp + h
                            sl = slice(s * 512, (s + 1) * 512)
                            nc.vector.tensor_scalar(
                                d0[:, sl], zt[:, sl], -1.0, 1.0, ALU.mult, ALU.add
                            )
                            nc.vector.tensor_mul(d1[:, sl], zt[:, sl], ht[:, sl])
                            # passes feeding an fp8 pass scan straight to
                            # the fp8 shadow state (fp32 scan accumulator;
                            # mx is only refreshed by pass n8 for the fp16
                            # passes) -- drops the Pool cast chain hop
                            to8 = p < n8
                            if s == 0:
                                init = zcar[:, c : c + 1]
                            elif last:
                                init = m32[:, c, s * 512 - 1 : s * 512]
                            elif to8:
                                init = m8[:, c, s * 512 : s * 512 + 1]
                            else:
                                init = mx[:, c, s * 512 : s * 512 + 1]
                            if last:
                                out_ap = m32[:, c, sl]
                            elif to8:
                                out_ap = m8[:, c, 1 + s * 512 : 1 + (s + 1) * 512]
                            else:
                                out_ap = mx[:, c, 1 + s * 512 : 1 + (s + 1) * 512]
                            nc.vector.tensor_tensor_scan(
                                out_ap, d0[:, sl], d1[:, sl], init, ALU.mult, ALU.add
                            )

                if p == npass - 2:
                    # single exchange, overlapping the final pass: my current
                    # final state -> all cores; I keep the previous core's.
                    cout = carryp.tile([128, KCH], FP32, tag="cout")
                    for c in range(KCH):
                        nc.vector.tensor_copy(
                            cout[:, c : c + 1], mx[:, c, BC : BC + 1]
                        )
                    nc.sync.dma_start(
                        cin[:].rearrange("o (p c) -> o p c", c=KCH), cout[:]
                    )
                    if not sim_nocc:
                        nc.gpsimd.collective_compute(
                            "AllGather",
                            ALU.bypass,
                            replica_groups=[list(range(NCORE))],
                            ins=[cin[:]],
                            outs=[gath9[1 : NCORE + 1, :]],
                        )
                    fcar = carryp.tile([128, KCH], FP32, tag="fcar")
                    nc.sync.dma_start(
                        fcar[:],
                        gath9[ds(pid, 1), :].rearrange("o (p c) -> o p c", c=KCH),
                    )

            def emit_out_group(tt):
                # fp16 transposes run at 1 cycle/row (fp32 needs 2); the
                # fp32 upcast happens in the DVE copy below
                yst = outs.tile([128, D], FP32, tag="yst")
                for c in range(KCH):
                    psy = pst2.tile([128, 128], FP16, tag="psy")
                    nc.tensor.transpose(
                        psy[:], m32[:, c, tt * 128 : (tt + 1) * 128], id16[:]
                    )
                    nc.vector.tensor_copy(yst[:, c * 128 : (c + 1) * 128], psy[:])
                nc.sync.dma_start(ys[tt * 128 : (tt + 1) * 128, :], yst[:])

            # ---------------- head-window fixup with the true carry --------
            # chunk-Jacobi with double-buffered state so all 4 feature chunks
            # pipeline per pass; tail-output groups (rows >= fixw are final
            # already) are emitted between passes so PE fills latency stalls
            nhead = max(1, fixw // 128)
            ngrp = BC // 128 - nhead
            mwA = fix.tile([128, KCH, fixw + 1], FP16, tag="mwA")
            mwB = fix.tile([128, KCH, fixw + 1], FP16, tag="mwB")
            for mwt in (mwA, mwB):
                for c in range(KCH):
                    nc.vector.tensor_copy(mwt[:, c, 0:1], fcar[:, c : c + 1])
            for c in range(KCH):
                nc.vector.tensor_copy(mwA[:, c, 1:fixw], m32[:, c, 0 : fixw - 1])
            emitted = 0
            for f in range(fixp):
                lastf = f == fixp - 1
                if fix_jacobi:
                    mwr = mwA if f % 2 == 0 else mwB
                    mww = mwB if f % 2 == 0 else mwA
                else:
                    mwr = mww = mwA
                # all four chunks of a gate type share one psum tile as
                # 64-col windows -> ONE 256-wide activation per gate type
                # (the small per-chunk acts were overhead-dominated)
                zw4 = fzh.tile([128, KCH, fixw], FP16, tag="w4")
                hw4 = fzh.tile([128, KCH, fixw], FP16, tag="w4")
                for base, dst, fn in ((0, zw4, AF.Sigmoid), (KCH, hw4, AF.Tanh)):
                    psw = ps2.tile([128, 512], FP32, tag="psg")
                    for c in range(KCH):
                        j = base + c
                        wsl = slice(c * fixw, (c + 1) * fixw)
                        nc.tensor.matmul(
                            psw[:, wsl],
                            id16[:],
                            azb[:, j, 0:fixw],
                            start=True,
                            stop=False,
                        )
                        for k in range(KCH):
                            nc.tensor.matmul(
                                psw[:, wsl],
                                usb[:, k, j * 128 : (j + 1) * 128],
                                mwr[:, k, 0:fixw],
                                start=False,
                                stop=(k == KCH - 1),
                            )
                    nc.scalar.activation(
                        dst[:], psw[:, 0 : KCH * fixw], fn
                    )
                d0w4 = fzh.tile([128, KCH, fixw], FP16, tag="w4")
                d1w4 = fzh.tile([128, KCH, fixw], FP16, tag="w4")
                nc.vector.tensor_scalar(
                    d0w4[:], zw4[:], -1.0, 1.0, ALU.mult, ALU.add
                )
                nc.vector.tensor_mul(d1w4[:], zw4[:], hw4[:])
                for c in range(KCH):
                    out_ap = m32[:, c, 0:fixw] if lastf else mww[:, c, 1 : fixw + 1]
                    nc.vector.tensor_tensor_scan(
                        out_ap,
                        d0w4[:, c, :],
                        d1w4[:, c, :],
                        fcar[:, c : c + 1],
                        ALU.mult,
                        ALU.add,
                    )
                ntail = (ngrp * (f + 1)) // fixp
                while emitted < ntail:
                    emit_out_group(nhead + emitted)
                    emitted += 1

            # head output (depends on the fixup-written rows)
            for tt in range(nhead):
                emit_out_group(tt)

        az2.release()
        consts.release()

    _split_multi_waits(nc)
    return nc



_CACHE = {}


def _make_runner(nc):
    """Single-core PJRT runner with a persistent jit cache (run_bass_via_pjrt
    builds a fresh closure per call, forcing a full recompile; this keeps the
    jitted body alive so repeat calls only pay transfer + execute)."""
    import jax
    from concourse import bass2jax

    bass2jax.install_neuronx_cc_hook()
    part_name = nc.partition_id_tensor.name if nc.partition_id_tensor else None
    in_names, out_names, out_avals = [], [], []
    for alloc in nc.m.functions[0].allocations:
        if not isinstance(alloc, mybir.MemoryLocationSet):
            continue
        name = alloc.memorylocations[0].name
        if alloc.kind == "ExternalInput":
            if name != part_name:
                in_names.append(name)
        elif alloc.kind == "ExternalOutput":
            out_names.append(name)
            out_avals.append(
                jax.core.ShapedArray(
                    tuple(alloc.tensor_shape), mybir.dt.np(alloc.dtype)
                )
            )
    n_params = len(in_names)
    all_names = in_names + out_names
    if part_name is not None:
        all_names = all_names + [part_name]
    all_names = tuple(all_names)
    donate = tuple(range(n_params, n_params + len(out_names)))

    def _body(*args):
        operands = list(args)
        if part_name is not None:
            operands.append(bass2jax.partition_id_tensor())
        outs = bass2jax._bass_exec_p.bind(
            *operands,
            out_avals=tuple(out_avals),
            in_names=all_names,
            out_names=tuple(out_names),
            lowering_input_output_aliases=(),
            sim_require_finite=True,
            sim_require_nnan=True,
            nc=nc,
        )
        return tuple(outs)

    jitted = jax.jit(_body, donate_argnums=donate, keep_unused=True)

    def run(in_map):
        args = [np.asarray(in_map[n]) for n in in_names[:n_params]]
        args += [np.zeros(a.shape, a.dtype) for a in out_avals]
        outs = jax.block_until_ready(jitted(*args))
        return {name: outs[i] for i, name in enumerate(out_names)}

    return run


def _host_prep(inputs):
    import ml_dtypes

    wp = np.concatenate(
        [np.asarray(inputs["Wz"], np.float32), np.asarray(inputs["Wh"], np.float32)],
        axis=1,
    ).astype(np.float16)
    up32 = np.concatenate(
        [np.asarray(inputs["Uz"], np.float32), np.asarray(inputs["Uh"], np.float32)],
        axis=1,
    )
    up = up32.astype(np.float16)
    up8 = up32.astype(ml_dtypes.float8_e4m3)

    def pack_pm(w):  # [D, DO] -> [128, KCH*DO] partition-major
        return np.ascontiguousarray(
            w.reshape(KCH, 128, DO).transpose(1, 0, 2).reshape(128, KCH * DO)
        )

    # wp j-chunked for streaming: [128, j, k, 128]
    wp = np.ascontiguousarray(
        wp.reshape(KCH, 128, JCH, 128).transpose(1, 2, 0, 3)
    )
    up = pack_pm(up)
    up8 = pack_pm(up8)
    bpack = (
        np.concatenate(
            [np.asarray(inputs["bz"], np.float32), np.asarray(inputs["bh"], np.float32)]
        )
        .reshape(JCH, 128)
        .T.copy()
        .astype(np.float32)
    )
    return {
        "wp": wp,
        "up": up,
        "up8": up8,
        "bp": bpack,
        "i16": np.eye(128, dtype=np.float16),
        "i32": np.eye(128, dtype=np.float32),
    }


def kernel(**inputs: np.ndarray) -> np.ndarray:
    """8-core block-Jacobi fixed point (default). Set MEMORY_KERNEL_MODE=single
    to fall back to the single-core blockwise kernel."""
    import os

    import jax

    x = np.ascontiguousarray(inputs["x"], dtype=np.float32)
    common = _host_prep(inputs)
    # Pin a real neuron device: with a CPU default device the bass_exec
    # primitive lowers to the MultiCoreSim fallback instead of hardware.
    dev = [d for d in jax.devices() if d.platform != "cpu"][0]

    single = os.environ.get("MEMORY_KERNEL_MODE", "").lower() == "single"
    last_exc = None
    for attempt in range(3):
        try:
            if single:
                if "nc1" not in _CACHE:
                    _CACHE["nc1"] = build_kernel()
                    _CACHE["runner1"] = _make_runner(_CACHE["nc1"])
                with jax.default_device(dev):
                    out = _CACHE["runner1"]({"x": x, **common})
                return np.ascontiguousarray(out["ys"])
            if "nc8" not in _CACHE:
                _CACHE["nc8"] = build_kernel8()
            xt_all = x.T.astype(np.float16)  # [D, T]
            ntb = BC // 512
            in_maps = []
            for c in range(NCORE):
                xc = xt_all[:, c * BC : (c + 1) * BC]  # [D, BC]
                # -> [128, tb, k, 512] partition-major
                xp = np.ascontiguousarray(
                    xc.reshape(KCH, 128, ntb, 512).transpose(1, 2, 0, 3)
                )
                in_maps.append({"xt": xp, **common})
            with jax.default_device(dev):
                res = run_bass_kernel_spmd(
                    _CACHE["nc8"], in_maps, core_ids=list(range(NCORE))
                )
            out = np.concatenate(
                [np.asarray(res.results[c]["ys"]) for c in range(NCORE)], axis=0
            )
            # transient first-exec glitches can yield garbage; |m_t| <= 1 by
            # construction (convex combination of tanh values), so re-run on
            # any non-finite or out-of-range output
            if not np.isfinite(out).all() or np.abs(out).max() > 1.5:
                raise RuntimeError("NRT transient: non-finite kernel output")
            return out
        except Exception as e:  # transient NRT device errors on first exec
            last_exc = e
            if (
                "UNRECOVERABLE" not in str(e)
                and "NRT" not in str(e)
                and "non-finite" not in str(e)
            ):
                raise
    raise last_exc


if __name__ == "__main__":
    rng = np.random.RandomState(0)
    ins = {
        "x": rng.randn(T, D).astype(np.float32),
        "Wz": (rng.randn(D, D) / np.sqrt(D)).astype(np.float32),
        "Uz": (rng.randn(D, D) / np.sqrt(D)).astype(np.float32),
        "bz": np.zeros(D, np.float32),
        "Wh": (rng.randn(D, D) / np.sqrt(D)).astype(np.float32),
        "Uh": (rng.randn(D, D) / np.sqrt(D)).astype(np.float32),
        "bh": np.zeros(D, np.float32),
    }
    out = kernel(**ins)
    print("out", out.shape, out.dtype, np.abs(out).max())

